# revision 1
# baseline (speedup 1.0000x reference)
"""AdaptiveGraphConv Trainium2 kernel: 8-core SPMD, data-parallel over B.

Reference computation (per (b,t) slice over V=25 nodes):
  th = theta(x), ph = phi(x)  (1x1 convs to INTER=32)
  A  = softmax(th @ ph / sqrt(INTER))   (V x V attention)
  out = A @ g(x)                        (g: 1x1 conv to C_OUT=128)
  BatchNorm2d (training stats over (B,T,V)) + affine.

Mapping: each core takes B/8=4 batches. Positions (t,v) are packed 5
t-slices (=125 positions) per PE "group"; scores for the 5 slices are
computed in one 125x125 matmul and block-diagonal-masked after exp.
Z (softmax denom) comes for free from a ones-column appended to g.
Normalize-then-transpose via an identity-rhs matmul gives the (C_OUT,
pos) layout; per-channel sum/sumsq accumulate in PSUM via a ones-lhsT
matmul. BN stats are all-reduced (2*128 floats) across the 8 cores and
applied as a per-channel affine fused into the output stream.

g_b is intentionally dropped: rows of A sum to 1, so +g_b[o] is a
constant per-channel shift that training-mode BN's mean subtraction
cancels exactly.
"""

import sys

sys.path.insert(0, "/opt/trn_rl_repo")

from contextlib import ExitStack

import numpy as np

from concourse import bacc, bass, mybir, tile
from concourse.bass_utils import run_bass_kernel_spmd

B, C_IN, T, V = 32, 64, 300, 25
C_OUT, INTER = 128, 32
EPS = 1e-5
NCORES = 8
BPC = B // NCORES            # batches per core
POS = BPC * T * V            # 30000 positions per core
G = 5                        # t-slices per PE group
GP = G * V                   # 125 positions per group
GW = 4                       # groups fused per wide chunk (500 positions)
WIDE = GW * GP               # 500
NG = POS // GP               # 240 groups per core
NT = B * T * V               # 240000 positions globally (BN denominator)
XCHUNK = 2500                # x stream chunk (cols); 12 chunks per core
OCHUNK = 2500                # output stream chunk; 12 chunks per core
SCALE = 1.0 / float(np.sqrt(INTER))

F32 = mybir.dt.float32
AF = mybir.ActivationFunctionType
ALU = mybir.AluOpType

_CACHE = {}


def _build(single_core=False):
    nc = bacc.Bacc(
        "TRN2",
        target_bir_lowering=False,
        debug=False,
        num_devices=1 if single_core else NCORES,
    )
    x_d = nc.dram_tensor("x", [C_IN, POS], F32, kind="ExternalInput")
    w2_d = nc.dram_tensor("w2", [C_IN, 2 * INTER], F32, kind="ExternalInput")
    gw_d = nc.dram_tensor("gw", [C_IN, C_OUT], F32, kind="ExternalInput")
    b2_d = nc.dram_tensor("b2", [2 * INTER, 1], F32, kind="ExternalInput")
    mask_d = nc.dram_tensor("mask", [GP, WIDE], F32, kind="ExternalInput")
    eye_d = nc.dram_tensor("eye", [GP, GP], F32, kind="ExternalInput")
    ones_d = nc.dram_tensor("ones", [GP, 1], F32, kind="ExternalInput")
    gb_d = nc.dram_tensor("gamma_beta", [1, 2 * C_OUT], F32, kind="ExternalInput")
    out_d = nc.dram_tensor("out", [C_OUT, POS], F32, kind="ExternalOutput")

    with tile.TileContext(nc) as tc, ExitStack() as ctx:
        const = ctx.enter_context(tc.tile_pool(name="const", bufs=1))
        stash_p = ctx.enter_context(tc.tile_pool(name="stash", bufs=1))
        xp = ctx.enter_context(tc.tile_pool(name="xp", bufs=2))
        wide_p = ctx.enter_context(tc.tile_pool(name="wide", bufs=2))
        work = ctx.enter_context(tc.tile_pool(name="work", bufs=3))
        outp = ctx.enter_context(tc.tile_pool(name="outp", bufs=2))
        ps_proj_p = ctx.enter_context(
            tc.tile_pool(name="psA", bufs=1, space="PSUM")
        )
        ps_s_p = ctx.enter_context(tc.tile_pool(name="psS", bufs=2, space="PSUM"))
        ps_g_p = ctx.enter_context(tc.tile_pool(name="psG", bufs=2, space="PSUM"))
        ps_o_p = ctx.enter_context(tc.tile_pool(name="psO", bufs=1, space="PSUM"))
        ps_y_p = ctx.enter_context(tc.tile_pool(name="psY", bufs=1, space="PSUM"))
        ps_st_p = ctx.enter_context(
            tc.tile_pool(name="psStat", bufs=1, space="PSUM")
        )
        dram = ctx.enter_context(tc.tile_pool(name="dram", bufs=1, space="DRAM"))

        w2 = const.tile([C_IN, 2 * INTER], F32)
        nc.sync.dma_start(w2[:], w2_d[:])
        gw = const.tile([C_IN, C_OUT], F32)
        nc.sync.dma_start(gw[:], gw_d[:])
        b2 = const.tile([2 * INTER, 1], F32)
        nc.sync.dma_start(b2[:], b2_d[:])
        mask = const.tile([GP, WIDE], F32)
        nc.sync.dma_start(mask[:], mask_d[:])
        eye = const.tile([GP, GP], F32)
        nc.sync.dma_start(eye[:], eye_d[:])
        ones = const.tile([GP, 1], F32)
        nc.sync.dma_start(ones[:], ones_d[:])
        gb = const.tile([1, 2 * C_OUT], F32)
        nc.sync.dma_start(gb[:], gb_d[:])

        stash = stash_p.tile([C_OUT, POS], F32)
        ps_stats = ps_st_p.tile([1, 2 * C_OUT], F32)

        gabs = 0
        for ci in range(POS // XCHUNK):
            x_sb = xp.tile([C_IN, XCHUNK], F32)
            nc.sync.dma_start(x_sb[:], x_d[:, ci * XCHUNK : (ci + 1) * XCHUNK])
            for wj in range(XCHUNK // WIDE):
                xoff = wj * WIDE
                ps_proj = ps_proj_p.tile([2 * INTER, WIDE], F32)
                nc.tensor.matmul(
                    ps_proj[:], w2[:], x_sb[:, xoff : xoff + WIDE],
                    start=True, stop=True,
                )
                th = wide_p.tile([INTER, WIDE], F32, tag="th")
                ph = wide_p.tile([INTER, WIDE], F32, tag="ph")
                nc.scalar.activation(
                    th[:], ps_proj[0:INTER, :], AF.Identity, bias=b2[0:INTER, :]
                )
                nc.scalar.activation(
                    ph[:], ps_proj[INTER : 2 * INTER, :], AF.Identity,
                    bias=b2[INTER : 2 * INTER, :],
                )
                ps_s = ps_s_p.tile([GP, WIDE], F32)
                for j in range(GW):
                    sl = slice(j * GP, (j + 1) * GP)
                    # scoresT[w, v] = sum_i ph[i, w] * th[i, v]
                    nc.tensor.matmul(
                        ps_s[:, sl], ph[:, sl], th[:, sl], start=True, stop=True
                    )
                pexp = wide_p.tile([GP, WIDE], F32, tag="pexp")
                nc.scalar.activation(pexp[:], ps_s[:], AF.Exp, scale=SCALE)
                pmT = wide_p.tile([GP, WIDE], F32, tag="pmT")
                nc.vector.tensor_mul(pmT[:], pexp[:], mask[:])
                for j in range(GW):
                    pos0 = ci * XCHUNK + xoff + j * GP
                    ps_g = ps_g_p.tile([GP, C_OUT], F32)
                    nc.tensor.matmul(
                        ps_g[:],
                        x_sb[:, xoff + j * GP : xoff + (j + 1) * GP],
                        gw[:],
                        start=True, stop=True,
                    )
                    g_sb = work.tile([GP, C_OUT + 1], F32, tag="g_sb")
                    nc.scalar.activation(g_sb[:, 0:C_OUT], ps_g[:], AF.Copy)
                    nc.gpsimd.memset(g_sb[:, C_OUT : C_OUT + 1], 1.0)
                    ps_o = ps_o_p.tile([GP, C_OUT + 1], F32)
                    nc.tensor.matmul(
                        ps_o[:], pmT[:, j * GP : (j + 1) * GP], g_sb[:],
                        start=True, stop=True,
                    )
                    rz = work.tile([GP, 1], F32, tag="rz")
                    nc.vector.reciprocal(rz[:], ps_o[:, C_OUT : C_OUT + 1])
                    stat_in = work.tile([GP, 2 * C_OUT], F32, tag="stat_in")
                    nc.vector.tensor_scalar_mul(
                        stat_in[:, 0:C_OUT], ps_o[:, 0:C_OUT], rz[:]
                    )
                    nc.scalar.square(
                        stat_in[:, C_OUT : 2 * C_OUT], stat_in[:, 0:C_OUT]
                    )
                    nc.tensor.matmul(
                        ps_stats[:], ones[:], stat_in[:],
                        start=(gabs == 0), stop=(gabs == NG - 1),
                    )
                    ps_y = ps_y_p.tile([C_OUT, GP], F32, tag="ps_y")
                    nc.tensor.matmul(
                        ps_y[:], stat_in[:, 0:C_OUT], eye[:], start=True, stop=True
                    )
                    nc.vector.tensor_copy(stash[:, pos0 : pos0 + GP], ps_y[:])
                    gabs += 1

        # ---- phase 2: BN stats all-reduce + per-channel affine coefs ----
        stats_sb = work.tile([1, 2 * C_OUT], F32, tag="stats_sb")
        nc.vector.tensor_copy(stats_sb[:], ps_stats[:])
        cc_in = dram.tile([1, 2 * C_OUT], F32)
        cc_out = dram.tile([1, 2 * C_OUT], F32)
        nc.sync.dma_start(cc_in[:], stats_sb[:])
        if single_core:
            nc.sync.dma_start(cc_out[:], cc_in[:])
        else:
            nc.gpsimd.collective_compute(
                "AllReduce",
                ALU.add,
                replica_groups=[list(range(NCORES))],
                ins=[cc_in.opt()],
                outs=[cc_out.opt()],
            )
        gstats = work.tile([1, 2 * C_OUT], F32, tag="gstats")
        nc.sync.dma_start(gstats[:], cc_out[:])
        # mean row, E[y^2] row
        mrow = work.tile([1, C_OUT], F32, tag="mrow")
        nc.vector.tensor_scalar_mul(mrow[:], gstats[:, 0:C_OUT], 1.0 / NT)
        vrow = work.tile([1, C_OUT], F32, tag="vrow")
        nc.vector.tensor_scalar_mul(vrow[:], gstats[:, C_OUT:], 1.0 / NT)
        m2row = work.tile([1, C_OUT], F32, tag="m2row")
        nc.scalar.square(m2row[:], mrow[:])
        nc.vector.tensor_sub(vrow[:], vrow[:], m2row[:])  # var = E[y^2]-mean^2
        nc.vector.tensor_scalar_add(vrow[:], vrow[:], float(EPS))
        srow = work.tile([1, C_OUT], F32, tag="srow")
        nc.scalar.activation(srow[:], vrow[:], AF.Sqrt)
        nc.vector.reciprocal(srow[:], srow[:])            # rstd
        nc.vector.tensor_mul(srow[:], srow[:], gb[:, 0:C_OUT])  # s = gamma*rstd
        crow = work.tile([1, C_OUT], F32, tag="crow")
        nc.vector.tensor_mul(crow[:], mrow[:], srow[:])
        nc.vector.tensor_sub(crow[:], gb[:, C_OUT:], crow[:])  # c = beta - mean*s
        # transpose (1,128) rows -> (128,1) cols via K=1 matmuls
        ps_sc = ps_y_p.tile([C_OUT, 2], F32, tag="ps_y")
        nc.tensor.matmul(ps_sc[:, 0:1], srow[:], ones[0:1, :], start=True, stop=True)
        nc.tensor.matmul(ps_sc[:, 1:2], crow[:], ones[0:1, :], start=True, stop=True)
        scol = work.tile([C_OUT, 1], F32, tag="scol")
        ccol = work.tile([C_OUT, 1], F32, tag="ccol")
        nc.vector.tensor_copy(scol[:], ps_sc[:, 0:1])
        nc.vector.tensor_copy(ccol[:], ps_sc[:, 1:2])

        # ---- phase 3: BN apply fused into output stream ----
        for ck in range(POS // OCHUNK):
            ob = outp.tile([C_OUT, OCHUNK], F32)
            nc.vector.tensor_scalar(
                ob[:],
                stash[:, ck * OCHUNK : (ck + 1) * OCHUNK],
                scol[:],
                ccol[:],
                ALU.mult,
                ALU.add,
            )
            nc.sync.dma_start(out_d[:, ck * OCHUNK : (ck + 1) * OCHUNK], ob[:])

    nc.compile()
    return nc


def _consts():
    mask = np.zeros((GP, WIDE), dtype=np.float32)
    for j in range(GW):
        for p in range(GP):
            s = p // V
            mask[p, j * GP + s * V : j * GP + (s + 1) * V] = 1.0
    # mask[p, j*GP+q] = 1 iff p//V == q//V; built above row-wise:
    # row p belongs to slice s=p//V -> cols of slice s in each group j.
    # But that sets mask[p, cols of slice s] which is exactly p//V==q//V. OK.
    eye = np.eye(GP, dtype=np.float32)
    ones = np.ones((GP, 1), dtype=np.float32)
    return mask, eye, ones


def kernel(x, theta_w, theta_b, phi_w, phi_b, g_w, g_b, bn_gamma, bn_beta):
    x = np.asarray(x, dtype=np.float32)
    if "nc" not in _CACHE:
        _CACHE["nc"] = _build()
    nc = _CACHE["nc"]

    w2 = np.concatenate(
        [np.asarray(theta_w).T, np.asarray(phi_w).T], axis=1
    ).astype(np.float32)  # (C_IN, 64)
    gwm = np.asarray(g_w).T.astype(np.float32).copy()  # (C_IN, C_OUT)
    b2 = np.concatenate([np.asarray(theta_b), np.asarray(phi_b)])[
        :, None
    ].astype(np.float32)
    mask, eye, ones = _consts()
    gb = np.concatenate([np.asarray(bn_gamma), np.asarray(bn_beta)])[
        None, :
    ].astype(np.float32)

    in_maps = []
    for c in range(NCORES):
        xs = (
            x[c * BPC : (c + 1) * BPC]
            .transpose(1, 0, 2, 3)
            .reshape(C_IN, POS)
            .copy()
        )
        in_maps.append(
            {
                "x": xs,
                "w2": w2,
                "gw": gwm,
                "b2": b2,
                "mask": mask,
                "eye": eye,
                "ones": ones,
                "gamma_beta": gb,
            }
        )

    res = run_bass_kernel_spmd(nc, in_maps, core_ids=list(range(NCORES)))
    out = np.empty((B, C_OUT, T, V), dtype=np.float32)
    for c in range(NCORES):
        oc = res.results[c]["out"]  # (C_OUT, POS), b-major positions
        out[c * BPC : (c + 1) * BPC] = (
            oc.reshape(C_OUT, BPC, T, V).transpose(1, 0, 2, 3)
        )
    return out



# revision 29
# speedup vs baseline: 2.2362x; 2.2362x over previous
"""AdaptiveGraphConv Trainium2 kernel: 8-core SPMD, data-parallel over B.

Reference computation (per (b,t) slice over V=25 nodes):
  th = theta(x)+b, ph = phi(x)+b   (1x1 convs to INTER=32)
  A  = softmax(th @ ph / sqrt(INTER))    (V x V attention)
  out = A @ g(x)                         (g: 1x1 conv to C_OUT=128)
  BatchNorm2d (training stats over (B,T,V)) + affine.

Mapping (each core: B/8=4 batches, POS=30000 positions, 240 groups of
125 positions = 5 t-slices):

  scores[v,w] = x^_v^T Q x^_w  where x^ = [x; 1; e_0..e_4; 1] (71 rows)
  and Q (71x71, host-precomputed) folds theta^T phi, both biases, AND
  the block-diagonal softmax mask (+169 on slice-indicator diagonal,
  -169 via the duplicate ones row) so cross-slice scores exp to ~1e-13.

  Per 500-col chunk (4 groups):
    R = Q x^                       (PE, N=500, bf16)
    S_j = R_j^T x^_j               (PE, 4x N=125, bf16) -> scoresT [w,v]
    P = exp(SCALE * S)             (ACT, psum->sbuf bf16)
    M1_j = [xT_j ones]^T P_j       (PE, 4x N=125, bf16): rows 0:64 = x@A
                                    unnormalized, row 64 = Z (softmax denom)
    Y  = gw^T M1[0:64]             (PE, N=500, f32r): [C_OUT, pos] unnorm
    ZB = ones_128 (x) M1[64]       (PE, N=500, f32r): Z broadcast
    stash = Y / ZB, accum col-sum  (DVE tensor_tensor_reduce)
  Sum of squares per channel accumulates on GpSimd from stash (SBUF only)
  interleaved with phase 1.  BN stats all-reduce ([128,2] floats), then
  per-channel affine fused into the fp32 output stream.

  g_b is dropped: rows of A sum to 1, so +g_b[o] is a per-channel shift
  that training-mode BN's mean subtraction cancels exactly.

Software pipeline: iteration k issues R(k), S(k-1), M1(k-2), Y/ZB/
norm(k-3), so no engine waits on same-iteration upstream results.
"""

import sys

sys.path.insert(0, "/opt/trn_rl_repo")

from contextlib import ExitStack

import numpy as np
import ml_dtypes

from concourse import bacc, bass, mybir, tile
from concourse.bass_utils import run_bass_kernel_spmd

B, C_IN, T, V = 32, 64, 300, 25
C_OUT, INTER = 128, 32
EPS = 1e-5
NCORES = 8
BPC = B // NCORES            # batches per core
POS = BPC * T * V            # 30000 positions per core
GP = 125                     # positions per PE group (5 t-slices)
NG = POS // GP               # 240 groups per core
CHUNK = 500                  # 4 groups per chunk
NCH = POS // CHUNK           # 60 chunks
SUP = 2500                   # superchunk (DMA granularity)
NSUP = POS // SUP            # 12
NT = B * T * V               # 240000 (BN denominator)
SCALE = 1.0 / float(np.sqrt(INTER))
XR = 71                      # x rows: 64 chans + ones + 5 indicators + ones
MR = 65                      # M1 rows: 64 chans + Z row
MBIG = 169.0                 # mask magnitude (exact in bf16)

F32 = mybir.dt.float32
BF16 = mybir.dt.bfloat16
F32R = mybir.dt.float32r
AF = mybir.ActivationFunctionType
ALU = mybir.AluOpType

_CACHE = {}


def _build(single_core=False):
    nc = bacc.Bacc(
        "TRN2",
        target_bir_lowering=False,
        debug=False,
        num_devices=1 if single_core else NCORES,
    )
    xe_d = nc.dram_tensor("xe", [XR, POS], BF16, kind="ExternalInput")
    xt_d = nc.dram_tensor("xt", [GP, NG * MR], BF16, kind="ExternalInput")
    qt_d = nc.dram_tensor("qt", [XR, XR], BF16, kind="ExternalInput")
    gw_d = nc.dram_tensor("gw", [C_IN, C_OUT], BF16, kind="ExternalInput")
    gb_d = nc.dram_tensor("gamma_beta", [C_OUT, 2], F32, kind="ExternalInput")
    out_d = nc.dram_tensor("out", [C_OUT, POS], F32, kind="ExternalOutput")

    with tile.TileContext(nc) as tc, ExitStack() as ctx:
        const = ctx.enter_context(tc.tile_pool(name="const", bufs=1))
        stash_p = ctx.enter_context(tc.tile_pool(name="stash", bufs=1))
        xsup_p = ctx.enter_context(tc.tile_pool(name="xsup", bufs=3))
        work = ctx.enter_context(tc.tile_pool(name="work", bufs=3))
        outp = ctx.enter_context(tc.tile_pool(name="outp", bufs=2))
        ps_r_p = ctx.enter_context(tc.tile_pool(name="psR", bufs=2, space="PSUM"))
        ps_s_p = ctx.enter_context(tc.tile_pool(name="psS", bufs=2, space="PSUM"))
        ps_m_p = ctx.enter_context(tc.tile_pool(name="psM", bufs=2, space="PSUM"))
        ps_y_p = ctx.enter_context(tc.tile_pool(name="psY", bufs=1, space="PSUM"))
        ps_b_p = ctx.enter_context(tc.tile_pool(name="psB", bufs=1, space="PSUM"))
        dram = ctx.enter_context(tc.tile_pool(name="dram", bufs=1, space="DRAM"))

        qt = const.tile([XR, XR], BF16)
        nc.sync.dma_start(qt[:], qt_d[:])
        gw = const.tile([C_IN, C_OUT], BF16)
        nc.sync.dma_start(gw[:], gw_d[:])
        gb = const.tile([C_OUT, 2], F32)
        nc.sync.dma_start(gb[:], gb_d[:])

        # all-ones; row 64 used as the K=1 stationary for the Z broadcast
        # (base partition must match m1's Z row at partition 64)
        onesrow = const.tile([MR, C_OUT], BF16)
        nc.gpsimd.memset(onesrow[:], 1.0)

        acc = const.tile([C_OUT, NCH], F32)
        sqacc = const.tile([C_OUT, NSUP], F32)
        sq_scratch = const.tile([C_OUT, SUP], BF16)

        stash = [stash_p.tile([C_OUT, SUP], BF16, name=f"st{s}") for s in range(NSUP)]

        # superchunk input tiles, prefetched 5 iterations ahead
        xe_t, xt_t = {}, {}

        def dma_sup(s):
            xe = xsup_p.tile([XR, SUP], BF16, tag="xe", name=f"xe{s}")
            nc.sync.dma_start(xe[:], xe_d[:, s * SUP : (s + 1) * SUP])
            xt = xsup_p.tile([GP, 20 * MR], BF16, tag="xt", name=f"xt{s}")
            nc.sync.dma_start(xt[:], xt_d[:, s * 20 * MR : (s + 1) * 20 * MR])
            xe_t[s], xt_t[s] = xe, xt

        dma_sup(0)

        r_sbs, pexps, m1s, rzs = {}, {}, {}, {}

        for k in range(NCH + 3):
            if k % 5 == 0 and k // 5 + 1 < NSUP:
                dma_sup(k // 5 + 1)
            # ---- stage D: Y(k-3), Z bcast+recip, normalize+stash+colsum
            # (emitted first: all inputs were produced in earlier iterations,
            # so the PE and DVE start each iteration without cross-engine
            # waits)
            c = k - 3
            if 0 <= c < NCH:
                m1 = m1s.pop(c)
                ps_b = ps_b_p.tile([C_OUT, CHUNK], F32)
                nc.tensor.matmul(
                    ps_b[:],
                    onesrow[C_IN : C_IN + 1, :],
                    m1[C_IN : C_IN + 1, :],
                    start=True, stop=True,
                )
                ps_y = ps_y_p.tile([C_OUT, CHUNK], F32)
                nc.tensor.matmul(
                    ps_y[:], gw[:], m1[0:C_IN, :], start=True, stop=True
                )
                zb = work.tile([C_OUT, CHUNK], BF16, tag="zb", name=f"zb{c}")
                with nc.allow_low_precision(reason="1/Z in bf16 is plenty"):
                    nc.vector.reciprocal(zb[:], ps_b[:])
                s_idx, soff = c // 5, (c % 5) * CHUNK
                nc.vector.scalar_tensor_tensor(
                    out=stash[s_idx][:, soff : soff + CHUNK],
                    in0=ps_y[:],
                    scalar=1.0,
                    in1=zb[:],
                    op0=ALU.mult,
                    op1=ALU.mult,
                    accum_out=acc[:, c : c + 1],
                )
            # ---- sum-of-squares (DVE), one superchunk per 5 iterations ----
            if k >= 8 and (k - 8) % 5 == 0:
                s = (k - 8) // 5
                nc.vector.scalar_tensor_tensor(
                    out=sq_scratch[:],
                    in0=stash[s][:],
                    scalar=1.0,
                    in1=stash[s][:],
                    op0=ALU.mult,
                    op1=ALU.mult,
                    accum_out=sqacc[:, s : s + 1],
                )
            # ---- stage A: R(k) = Q @ x^ ----
            if k < NCH:
                xe = xe_t[k // 5]
                off = (k % 5) * CHUNK
                ps_r = ps_r_p.tile([XR, CHUNK], F32)
                nc.tensor.matmul(
                    ps_r[:], qt[:], xe[:, off : off + CHUNK], start=True, stop=True
                )
                r_sb = work.tile([XR, CHUNK], BF16, tag="r", name=f"r{k}")
                nc.scalar.activation(r_sb[:], ps_r[:], AF.Copy)
                r_sbs[k] = r_sb
            # ---- stage B: scoresT(k-1) + exp ----
            c = k - 1
            if 0 <= c < NCH:
                xe = xe_t[c // 5]
                off = (c % 5) * CHUNK
                r_sb = r_sbs.pop(c)
                ps_s = ps_s_p.tile([GP, CHUNK], F32)
                for j in range(4):
                    sl = slice(j * GP, (j + 1) * GP)
                    nc.tensor.matmul(
                        ps_s[:, sl],
                        r_sb[:, sl],
                        xe[:, off + j * GP : off + (j + 1) * GP],
                        start=True, stop=True,
                    )
                pexp = work.tile([GP, CHUNK], BF16, tag="pexp", name=f"p{c}")
                nc.scalar.activation(pexp[:], ps_s[:], AF.Exp, scale=SCALE)
                pexps[c] = pexp
            # ---- stage C: M1(k-2) = [xT ones]^T @ P (row 64 = Z) ----
            c = k - 2
            if 0 <= c < NCH:
                xt = xt_t[c // 5]
                pexp = pexps.pop(c)
                ps_m = ps_m_p.tile([MR, CHUNK], F32)
                for j in range(4):
                    gg = (c % 5) * 4 + j
                    nc.tensor.matmul(
                        ps_m[:, j * GP : (j + 1) * GP],
                        xt[:, gg * MR : (gg + 1) * MR],
                        pexp[:, j * GP : (j + 1) * GP],
                        start=True, stop=True,
                    )
                m1 = work.tile([MR, CHUNK], BF16, tag="m1", name=f"m{c}")
                nc.scalar.activation(m1[:], ps_m[:], AF.Copy)
                m1s[c] = m1

        nc.vector.scalar_tensor_tensor(
            out=sq_scratch[:],
            in0=stash[NSUP - 1][:],
            scalar=1.0,
            in1=stash[NSUP - 1][:],
            op0=ALU.mult,
            op1=ALU.mult,
            accum_out=sqacc[:, NSUP - 1 : NSUP],
        )

        # ---- phase 2: BN stats all-reduce + per-channel affine coefs ----
        cc_sb = work.tile([C_OUT, 2], F32, tag="ccsb")
        nc.vector.tensor_reduce(
            cc_sb[:, 0:1], acc[:], mybir.AxisListType.X, ALU.add
        )
        nc.vector.tensor_reduce(
            cc_sb[:, 1:2], sqacc[:], mybir.AxisListType.X, ALU.add
        )
        cc_in = dram.tile([C_OUT, 2], F32)
        cc_out = dram.tile([C_OUT, 2], F32)
        nc.sync.dma_start(cc_in[:], cc_sb[:])
        if single_core:
            nc.sync.dma_start(cc_out[:], cc_in[:])
        else:
            nc.gpsimd.collective_compute(
                "AllReduce",
                ALU.add,
                replica_groups=[list(range(NCORES))],
                ins=[cc_in.opt()],
                outs=[cc_out.opt()],
            )
        gstats = work.tile([C_OUT, 2], F32, tag="gstats")
        nc.sync.dma_start(gstats[:], cc_out[:])
        mcol = work.tile([C_OUT, 1], F32, tag="mcol")
        nc.vector.tensor_scalar_mul(mcol[:], gstats[:, 0:1], 1.0 / NT)
        vcol = work.tile([C_OUT, 1], F32, tag="vcol")
        nc.vector.tensor_scalar_mul(vcol[:], gstats[:, 1:2], 1.0 / NT)
        m2col = work.tile([C_OUT, 1], F32, tag="m2col")
        nc.scalar.square(m2col[:], mcol[:])
        nc.vector.tensor_sub(vcol[:], vcol[:], m2col[:])  # var = E[y^2]-mean^2
        nc.vector.tensor_scalar_add(vcol[:], vcol[:], float(EPS))
        sdcol = work.tile([C_OUT, 1], F32, tag="sdcol")
        nc.scalar.activation(sdcol[:], vcol[:], AF.Sqrt)
        scol = work.tile([C_OUT, 1], F32, tag="scol")
        nc.vector.reciprocal(scol[:], sdcol[:])            # rstd
        nc.vector.tensor_mul(scol[:], scol[:], gb[:, 0:1])  # s = gamma*rstd
        ccol = work.tile([C_OUT, 1], F32, tag="ccol")
        nc.vector.tensor_mul(ccol[:], mcol[:], scol[:])
        nc.vector.tensor_sub(ccol[:], gb[:, 1:2], ccol[:])  # c = beta - mean*s

        # ---- phase 3: BN affine fused into fp32 output stream ----
        for s in range(NSUP):
            ob = outp.tile([C_OUT, SUP], F32, tag="ob", name=f"ob{s}")
            if s % 2 == 0:
                nc.scalar.activation(
                    ob[:], stash[s][:], AF.Identity, bias=ccol[:], scale=scol[:]
                )
            else:
                nc.vector.tensor_scalar(
                    ob[:], stash[s][:], scol[:], ccol[:], ALU.mult, ALU.add
                )
            nc.sync.dma_start(out_d[:, s * SUP : (s + 1) * SUP], ob[:])

    nc.compile()
    return nc


def _host_prep(theta_w, theta_b, phi_w, phi_b, g_w, bn_gamma, bn_beta):
    th_hat = np.concatenate(
        [np.asarray(theta_w), np.asarray(theta_b)[:, None]], axis=1
    ).astype(np.float64)  # [32, 65]
    ph_hat = np.concatenate(
        [np.asarray(phi_w), np.asarray(phi_b)[:, None]], axis=1
    ).astype(np.float64)
    q = np.zeros((XR, XR), dtype=np.float64)
    q[:65, :65] = th_hat.T @ ph_hat
    for s in range(5):
        q[65 + s, 65 + s] = MBIG
    q[70, 70] = -MBIG
    qt = q.T.astype(ml_dtypes.bfloat16)
    gwT = np.asarray(g_w).T.astype(ml_dtypes.bfloat16).copy()  # [64, 128]
    gbmat = np.stack(
        [np.asarray(bn_gamma), np.asarray(bn_beta)], axis=1
    ).astype(np.float32)  # [128, 2]
    return qt, gwT, gbmat


def _per_core_inputs(xc):
    # xc: [64, POS] float32 for this core (b-major positions)
    pos = np.arange(POS)
    ind = ((pos // V) % 5)[None, :] == np.arange(5)[:, None]  # [5, POS]
    xe = np.concatenate(
        [
            xc,
            np.ones((1, POS), np.float32),
            ind.astype(np.float32),
            np.ones((1, POS), np.float32),
        ],
        axis=0,
    ).astype(ml_dtypes.bfloat16)  # [71, POS]
    xt = np.concatenate(
        [
            xc.reshape(C_IN, NG, GP).transpose(2, 1, 0),  # [125, 240, 64]
            np.ones((GP, NG, 1), np.float32),
        ],
        axis=2,
    ).reshape(GP, NG * MR).astype(ml_dtypes.bfloat16)
    return xe, xt


def kernel(x, theta_w, theta_b, phi_w, phi_b, g_w, g_b, bn_gamma, bn_beta):
    x = np.asarray(x, dtype=np.float32)
    if "nc" not in _CACHE:
        _CACHE["nc"] = _build()
    nc = _CACHE["nc"]

    qt, gwT, gbmat = _host_prep(
        theta_w, theta_b, phi_w, phi_b, g_w, bn_gamma, bn_beta
    )

    in_maps = []
    for c in range(NCORES):
        xc = (
            x[c * BPC : (c + 1) * BPC]
            .transpose(1, 0, 2, 3)
            .reshape(C_IN, POS)
        )
        xe, xt = _per_core_inputs(xc)
        in_maps.append(
            {
                "xe": xe,
                "xt": xt,
                "qt": qt,
                "gw": gwT,
                "gamma_beta": gbmat,
            }
        )

    res = run_bass_kernel_spmd(nc, in_maps, core_ids=list(range(NCORES)))
    out = np.empty((B, C_OUT, T, V), dtype=np.float32)
    for c in range(NCORES):
        oc = res.results[c]["out"]  # (C_OUT, POS), b-major positions
        out[c * BPC : (c + 1) * BPC] = (
            oc.reshape(C_OUT, BPC, T, V).transpose(1, 0, 2, 3)
        )
    return out


# revision 42
# speedup vs baseline: 2.4954x; 1.1159x over previous
"""AdaptiveGraphConv Trainium2 kernel: 8-core SPMD, data-parallel over B.

Reference computation (per (b,t) slice over V=25 nodes):
  th = theta(x)+b, ph = phi(x)+b   (1x1 convs to INTER=32)
  A  = softmax(th @ ph / sqrt(INTER))    (V x V attention)
  out = A @ g(x)                         (g: 1x1 conv to C_OUT=128)
  BatchNorm2d (training stats over (B,T,V)) + affine.

Mapping (each core: B/8=4 batches, POS=30000 positions, 240 groups of
125 positions = 5 t-slices):

  scores[v,w] = x^_v^T Q x^_w  where x^ = [x; 1; e_0..e_4; 1] (71 rows)
  and Q (71x71, host-precomputed) folds theta^T phi, both biases, AND
  the block-diagonal softmax mask (+169 on slice-indicator diagonal,
  -169 via the duplicate ones row) so cross-slice scores exp to ~1e-13.

  Per 500-col chunk (4 groups):
    R = Q x^                       (PE, N=500, bf16)
    S_j = R_j^T x^_j               (PE, 4x N=125, bf16) -> scoresT [w,v]
    P = exp(SCALE * S)             (ACT, psum->sbuf bf16)
    M1_j = [xT_j ones]^T P_j       (PE, 4x N=125, bf16): rows 0:64 = x@A
                                    unnormalized, row 64 = Z (softmax denom)
    Y  = gw^T M1[0:64]             (PE, N=500, bf16): [C_OUT, pos] unnorm
    ZB = ones (x) M1[64]           (PE, K=1 N=500): Z broadcast to PSUM
    zb = 1/ZB                      (DVE reciprocal, bf16 out)
    stash = Y*zb, accum col-sum    (DVE scalar_tensor_tensor)
  Sum of squares per channel accumulates via DVE scalar_tensor_tensor
  over stash superchunks interleaved with phase 1.  BN stats all-reduce
  ([128,2] floats), then per-channel affine fused into the fp32 output
  stream.

  g_b is dropped: rows of A sum to 1, so +g_b[o] is a per-channel shift
  that training-mode BN's mean subtraction cancels exactly.

Software pipeline: iteration k issues R(k), S(k-1), M1(k-2), Y/ZB/
norm(k-3), so no engine waits on same-iteration upstream results.
"""

import sys

sys.path.insert(0, "/opt/trn_rl_repo")

from contextlib import ExitStack

import numpy as np
import ml_dtypes

from concourse import bacc, bass, mybir, tile
from concourse.bass_utils import run_bass_kernel_spmd

B, C_IN, T, V = 32, 64, 300, 25
C_OUT, INTER = 128, 32
EPS = 1e-5
NCORES = 8
BPC = B // NCORES            # batches per core
POS = BPC * T * V            # 30000 positions per core
GP = 125                     # positions per PE group (5 t-slices)
NG = POS // GP               # 240 groups per core
CHUNK = 500                  # 4 groups per chunk
NCH = POS // CHUNK           # 60 chunks
SUP = 2500                   # superchunk (DMA granularity)
NSUP = POS // SUP            # 12
NT = B * T * V               # 240000 (BN denominator)
SCALE = 1.0 / float(np.sqrt(INTER))
XR = 71                      # x rows: 64 chans + ones + 5 indicators + ones
MR = 65                      # M1 rows: 64 chans + Z row
MBIG = 169.0                 # mask magnitude (exact in bf16)

F32 = mybir.dt.float32
BF16 = mybir.dt.bfloat16
F32R = mybir.dt.float32r
AF = mybir.ActivationFunctionType
ALU = mybir.AluOpType

_CACHE = {}


def _build(single_core=False):
    nc = bacc.Bacc(
        "TRN2",
        target_bir_lowering=False,
        debug=False,
        num_devices=1 if single_core else NCORES,
    )
    xe_d = nc.dram_tensor("xe", [XR, POS], BF16, kind="ExternalInput")
    xt_d = nc.dram_tensor("xt", [GP, NG * MR], BF16, kind="ExternalInput")
    qt_d = nc.dram_tensor("qt", [XR, XR], BF16, kind="ExternalInput")
    gw_d = nc.dram_tensor("gw", [C_IN, C_OUT], BF16, kind="ExternalInput")
    gb_d = nc.dram_tensor("gamma_beta", [C_OUT, 2], F32, kind="ExternalInput")
    out_d = nc.dram_tensor("out", [C_OUT, POS], F32, kind="ExternalOutput")

    with tile.TileContext(nc) as tc, ExitStack() as ctx:
        const = ctx.enter_context(tc.tile_pool(name="const", bufs=1))
        stash_p = ctx.enter_context(tc.tile_pool(name="stash", bufs=1))
        xsup_p = ctx.enter_context(tc.tile_pool(name="xsup", bufs=3))
        work = ctx.enter_context(tc.tile_pool(name="work", bufs=3))
        outp = ctx.enter_context(tc.tile_pool(name="outp", bufs=2))
        ps_r_p = ctx.enter_context(tc.tile_pool(name="psR", bufs=2, space="PSUM"))
        ps_s_p = ctx.enter_context(tc.tile_pool(name="psS", bufs=2, space="PSUM"))
        ps_m_p = ctx.enter_context(tc.tile_pool(name="psM", bufs=2, space="PSUM"))
        ps_y_p = ctx.enter_context(tc.tile_pool(name="psY", bufs=1, space="PSUM"))
        ps_b_p = ctx.enter_context(tc.tile_pool(name="psB", bufs=1, space="PSUM"))
        dram = ctx.enter_context(tc.tile_pool(name="dram", bufs=1, space="DRAM"))

        qt = const.tile([XR, XR], BF16)
        nc.sync.dma_start(qt[:], qt_d[:])
        gw = const.tile([C_IN, C_OUT], BF16)
        nc.sync.dma_start(gw[:], gw_d[:])
        gb = const.tile([C_OUT, 2], F32)
        nc.sync.dma_start(gb[:], gb_d[:])

        # K=1 stationary for broadcasting the 1/Z row (partition 0)
        ones0 = const.tile([1, C_OUT], BF16)
        nc.gpsimd.memset(ones0[:], 1.0)

        acc = const.tile([C_OUT, NCH], F32)
        sqacc = const.tile([C_OUT, NSUP], F32)
        sq_scratch = const.tile([C_OUT, SUP], BF16)

        stash = [stash_p.tile([C_OUT, SUP], BF16, name=f"st{s}") for s in range(NSUP)]

        # superchunk input tiles, prefetched 5 iterations ahead
        xe_t, xt_t = {}, {}

        def dma_sup(s):
            xe = xsup_p.tile([XR, SUP], BF16, tag="xe", name=f"xe{s}")
            nc.sync.dma_start(xe[:], xe_d[:, s * SUP : (s + 1) * SUP])
            xt = xsup_p.tile([GP, 20 * MR], BF16, tag="xt", name=f"xt{s}")
            nc.sync.dma_start(xt[:], xt_d[:, s * 20 * MR : (s + 1) * 20 * MR])
            xe_t[s], xt_t[s] = xe, xt

        dma_sup(0)

        r_sbs, pexps, m1s, rzs = {}, {}, {}, {}

        for k in range(NCH + 3):
            if k % 5 == 0 and k // 5 + 1 < NSUP:
                dma_sup(k // 5 + 1)
            # ---- stage D: Y(k-3), Z bcast+recip, normalize+stash+colsum
            # (emitted first: all inputs were produced in earlier iterations,
            # so the PE and DVE start each iteration without cross-engine
            # waits)
            c = k - 3
            if 0 <= c < NCH:
                m1 = m1s.pop(c)
                rzrow = rzs.pop(c)
                ps_b = ps_b_p.tile([C_OUT, CHUNK], F32)
                nc.tensor.matmul(
                    ps_b[:], ones0[:], rzrow[:], start=True, stop=True
                )
                ps_y = ps_y_p.tile([C_OUT, CHUNK], F32)
                nc.tensor.matmul(
                    ps_y[:], gw[:], m1[0:C_IN, :], start=True, stop=True
                )
                zb = work.tile([C_OUT, CHUNK], BF16, tag="zb", name=f"zb{c}")
                nc.vector.tensor_copy(zb[:], ps_b[:])
                s_idx, soff = c // 5, (c % 5) * CHUNK
                nc.vector.scalar_tensor_tensor(
                    out=stash[s_idx][:, soff : soff + CHUNK],
                    in0=ps_y[:],
                    scalar=1.0,
                    in1=zb[:],
                    op0=ALU.mult,
                    op1=ALU.mult,
                    accum_out=acc[:, c : c + 1],
                )
            # ---- sum-of-squares (DVE), one superchunk per 5 iterations ----
            if k >= 8 and (k - 8) % 5 == 0:
                s = (k - 8) // 5
                nc.vector.scalar_tensor_tensor(
                    out=sq_scratch[:],
                    in0=stash[s][:],
                    scalar=1.0,
                    in1=stash[s][:],
                    op0=ALU.mult,
                    op1=ALU.mult,
                    accum_out=sqacc[:, s : s + 1],
                )
            # ---- stage A: R(k) = Q @ x^ ----
            if k < NCH:
                xe = xe_t[k // 5]
                off = (k % 5) * CHUNK
                ps_r = ps_r_p.tile([XR, CHUNK], F32)
                nc.tensor.matmul(
                    ps_r[:], qt[:], xe[:, off : off + CHUNK], start=True, stop=True
                )
                r_sb = work.tile([XR, CHUNK], BF16, tag="r", name=f"r{k}")
                nc.scalar.activation(r_sb[:], ps_r[:], AF.Copy)
                r_sbs[k] = r_sb
            # ---- stage B: scoresT(k-1) + exp ----
            c = k - 1
            if 0 <= c < NCH:
                xe = xe_t[c // 5]
                off = (c % 5) * CHUNK
                r_sb = r_sbs.pop(c)
                ps_s = ps_s_p.tile([GP, CHUNK], F32)
                for j in range(4):
                    sl = slice(j * GP, (j + 1) * GP)
                    nc.tensor.matmul(
                        ps_s[:, sl],
                        r_sb[:, sl],
                        xe[:, off + j * GP : off + (j + 1) * GP],
                        start=True, stop=True,
                    )
                pexp = work.tile([GP, CHUNK], BF16, tag="pexp", name=f"p{c}")
                nc.scalar.activation(pexp[:], ps_s[:], AF.Exp, scale=SCALE)
                pexps[c] = pexp
            # ---- stage C: M1(k-2) = [xT ones]^T @ P (row 64 = Z) ----
            c = k - 2
            if 0 <= c < NCH:
                xt = xt_t[c // 5]
                pexp = pexps.pop(c)
                ps_m = ps_m_p.tile([MR, CHUNK], F32)
                for j in range(4):
                    gg = (c % 5) * 4 + j
                    nc.tensor.matmul(
                        ps_m[:, j * GP : (j + 1) * GP],
                        xt[:, gg * MR : (gg + 1) * MR],
                        pexp[:, j * GP : (j + 1) * GP],
                        start=True, stop=True,
                    )
                m1 = work.tile([MR, CHUNK], BF16, tag="m1", name=f"m{c}")
                nc.vector.tensor_copy(m1[:], ps_m[:])
                m1s[c] = m1
                # 1/Z row via exp(-ln(Z)) on the ACT table engine
                lnz = work.tile([1, CHUNK], F32, tag="lnz", name=f"ln{c}")
                nc.scalar.activation(
                    lnz[:], ps_m[C_IN : C_IN + 1, :], AF.Ln
                )
                rzrow = work.tile([1, CHUNK], BF16, tag="rzr", name=f"rr{c}")
                nc.scalar.activation(rzrow[:], lnz[:], AF.Exp, scale=-1.0)
                rzs[c] = rzrow

        nc.vector.scalar_tensor_tensor(
            out=sq_scratch[:],
            in0=stash[NSUP - 1][:],
            scalar=1.0,
            in1=stash[NSUP - 1][:],
            op0=ALU.mult,
            op1=ALU.mult,
            accum_out=sqacc[:, NSUP - 1 : NSUP],
        )

        # ---- phase 2: BN stats all-reduce + per-channel affine coefs ----
        cc_sb = work.tile([C_OUT, 2], F32, tag="ccsb")
        nc.vector.tensor_reduce(
            cc_sb[:, 0:1], acc[:], mybir.AxisListType.X, ALU.add
        )
        nc.vector.tensor_reduce(
            cc_sb[:, 1:2], sqacc[:], mybir.AxisListType.X, ALU.add
        )
        cc_in = dram.tile([C_OUT, 2], F32)
        cc_out = dram.tile([C_OUT, 2], F32)
        nc.sync.dma_start(cc_in[:], cc_sb[:])
        if single_core:
            nc.sync.dma_start(cc_out[:], cc_in[:])
        else:
            nc.gpsimd.collective_compute(
                "AllReduce",
                ALU.add,
                replica_groups=[list(range(NCORES))],
                ins=[cc_in.opt()],
                outs=[cc_out.opt()],
            )
        gstats = work.tile([C_OUT, 2], F32, tag="gstats")
        nc.sync.dma_start(gstats[:], cc_out[:])
        mcol = work.tile([C_OUT, 1], F32, tag="mcol")
        nc.vector.tensor_scalar_mul(mcol[:], gstats[:, 0:1], 1.0 / NT)
        vcol = work.tile([C_OUT, 1], F32, tag="vcol")
        nc.vector.tensor_scalar_mul(vcol[:], gstats[:, 1:2], 1.0 / NT)
        m2col = work.tile([C_OUT, 1], F32, tag="m2col")
        nc.scalar.square(m2col[:], mcol[:])
        nc.vector.tensor_sub(vcol[:], vcol[:], m2col[:])  # var = E[y^2]-mean^2
        nc.vector.tensor_scalar_add(vcol[:], vcol[:], float(EPS))
        sdcol = work.tile([C_OUT, 1], F32, tag="sdcol")
        nc.scalar.activation(sdcol[:], vcol[:], AF.Sqrt)
        scol = work.tile([C_OUT, 1], F32, tag="scol")
        nc.vector.reciprocal(scol[:], sdcol[:])            # rstd
        nc.vector.tensor_mul(scol[:], scol[:], gb[:, 0:1])  # s = gamma*rstd
        ccol = work.tile([C_OUT, 1], F32, tag="ccol")
        nc.vector.tensor_mul(ccol[:], mcol[:], scol[:])
        nc.vector.tensor_sub(ccol[:], gb[:, 1:2], ccol[:])  # c = beta - mean*s

        # ---- phase 3: BN affine fused into fp32 output stream ----
        for s in range(NSUP):
            ob = outp.tile([C_OUT, SUP], F32, tag="ob", name=f"ob{s}")
            if s % 2 == 0:
                nc.scalar.activation(
                    ob[:], stash[s][:], AF.Identity, bias=ccol[:], scale=scol[:]
                )
            else:
                nc.vector.tensor_scalar(
                    ob[:], stash[s][:], scol[:], ccol[:], ALU.mult, ALU.add
                )
            nc.sync.dma_start(out_d[:, s * SUP : (s + 1) * SUP], ob[:])

    nc.compile()
    return nc


def _host_prep(theta_w, theta_b, phi_w, phi_b, g_w, bn_gamma, bn_beta):
    th_hat = np.concatenate(
        [np.asarray(theta_w), np.asarray(theta_b)[:, None]], axis=1
    ).astype(np.float64)  # [32, 65]
    ph_hat = np.concatenate(
        [np.asarray(phi_w), np.asarray(phi_b)[:, None]], axis=1
    ).astype(np.float64)
    q = np.zeros((XR, XR), dtype=np.float64)
    q[:65, :65] = th_hat.T @ ph_hat
    for s in range(5):
        q[65 + s, 65 + s] = MBIG
    q[70, 70] = -MBIG
    qt = q.T.astype(ml_dtypes.bfloat16)
    gwT = np.asarray(g_w).T.astype(ml_dtypes.bfloat16).copy()  # [64, 128]
    gbmat = np.stack(
        [np.asarray(bn_gamma), np.asarray(bn_beta)], axis=1
    ).astype(np.float32)  # [128, 2]
    return qt, gwT, gbmat


def _per_core_inputs(xc):
    # xc: [64, POS] float32 for this core (b-major positions)
    pos = np.arange(POS)
    ind = ((pos // V) % 5)[None, :] == np.arange(5)[:, None]  # [5, POS]
    xe = np.concatenate(
        [
            xc,
            np.ones((1, POS), np.float32),
            ind.astype(np.float32),
            np.ones((1, POS), np.float32),
        ],
        axis=0,
    ).astype(ml_dtypes.bfloat16)  # [71, POS]
    xt = np.concatenate(
        [
            xc.reshape(C_IN, NG, GP).transpose(2, 1, 0),  # [125, 240, 64]
            np.ones((GP, NG, 1), np.float32),
        ],
        axis=2,
    ).reshape(GP, NG * MR).astype(ml_dtypes.bfloat16)
    return xe, xt


def kernel(x, theta_w, theta_b, phi_w, phi_b, g_w, g_b, bn_gamma, bn_beta):
    x = np.asarray(x, dtype=np.float32)
    if "nc" not in _CACHE:
        _CACHE["nc"] = _build()
    nc = _CACHE["nc"]

    qt, gwT, gbmat = _host_prep(
        theta_w, theta_b, phi_w, phi_b, g_w, bn_gamma, bn_beta
    )

    in_maps = []
    for c in range(NCORES):
        xc = (
            x[c * BPC : (c + 1) * BPC]
            .transpose(1, 0, 2, 3)
            .reshape(C_IN, POS)
        )
        xe, xt = _per_core_inputs(xc)
        in_maps.append(
            {
                "xe": xe,
                "xt": xt,
                "qt": qt,
                "gw": gwT,
                "gamma_beta": gbmat,
            }
        )

    res = run_bass_kernel_spmd(nc, in_maps, core_ids=list(range(NCORES)))
    out = np.empty((B, C_OUT, T, V), dtype=np.float32)
    for c in range(NCORES):
        oc = res.results[c]["out"]  # (C_OUT, POS), b-major positions
        out[c * BPC : (c + 1) * BPC] = (
            oc.reshape(C_OUT, BPC, T, V).transpose(1, 0, 2, 3)
        )
    return out
